# revision 9
# baseline (speedup 1.0000x reference)
"""Trainium2 Bass kernel for nn_Corr_30099130811040.

Reference computation (N=2, C=256, H=W=64, nclass=21, h_out=w_out=128):
  feat = feature_in (identity resize)
  f1 = w1 @ feat + b1          [N, 21, 4096]
  f2 = w2 @ feat + b2          [N, 21, 4096]
  out_r = bilinear_down(out)   [N, 21, 64, 64] -> out_temp [N, 21, 4096]
  corr_map = softmax(f1^T f2 / sqrt(21), axis=-1)   [N, 4096, 4096]
  samp = corr_map[:, index, :] -> bilinear_up to 128x128, minmax-norm, > 0.5
  corr_out = out_temp @ corr_map                    [N, 21, 4096]

Sharding: data-parallel over N x p-rows. 8 cores; cores 0-3 own batch 0,
cores 4-7 own batch 1. Within a group of 4 cores, each owns 1024 rows (p)
of corr_map (flash-style: rows never hit HBM) and 32 of the 128 sampled
row-slots. Host gathers: partial corr_out summed over the 4 cores of a
group; sampled boolean maps concatenated + relaid out.

Algebraic simplifications:
  - softmax normalization folded: E = exp(S) rows are used unnormalized;
    corr_out uses out_temp^T * (1/rowsum) as matmul weights, and the
    minmax-threshold of the sampled rows is invariant to positive row
    scaling (no max-subtraction either: |S| stays small).
  - 1/sqrt(21) folded into w1/b1 on host.
  - (x-mn)/(mx-mn) > 0.5  <=>  x > (mn+mx)/2.

Perf notes: all DMAs are shaped for large contiguous runs (HW DGE cost is
per-run, ~10-25 ns each); host passes pre-tiled input layouts; on-chip
transposes use the PE; the one strided gather (cmT) is split across 8 DMA
queues.
"""

import numpy as np
from contextlib import ExitStack

import concourse.bass as bass
import concourse.tile as tile
from concourse import bacc, mybir
from concourse.bass_utils import run_bass_kernel_spmd

F32 = mybir.dt.float32
BF16 = mybir.dt.bfloat16
U8 = mybir.dt.uint8
AF = mybir.ActivationFunctionType
ALU = mybir.AluOpType

# problem dims (hardcoded per contract)
N, C, H, W = 2, 256, 64, 64
HW = H * W                    # 4096
NCLS = 21
HO = WO = 128
M = 128                       # sampled rows per batch element
NCORES = 8
GROUPS = 4                    # cores per batch element
PPC = HW // GROUPS            # 1024 p-rows per core
SPC = M // GROUPS             # 32 sample slots per core
NBLK = PPC // 128             # 8 p-blocks of 128 rows per core
KT = C // 128                 # 2 contraction tiles of 128
NQ = HW // 512                # 8 free-dim chunks


def _resize_matrix(out_size: int, in_size: int) -> np.ndarray:
    """Bilinear align_corners=True interpolation matrix [out_size, in_size]."""
    ys = np.linspace(0.0, in_size - 1.0, out_size)
    y0 = np.floor(ys).astype(np.int64)
    y1 = np.minimum(y0 + 1, in_size - 1)
    wy = ys - y0
    R = np.zeros((out_size, in_size), np.float64)
    R[np.arange(out_size), y0] += 1.0 - wy
    R[np.arange(out_size), y1] += wy
    return R.astype(np.float32)


def _build_nc(reps: int = 1, level: int = 5):
    nc = bacc.Bacc()

    # all host-side pre-tiled layouts; every load is contiguous per partition
    feat_d = nc.dram_tensor("feat", [128, KT, HW], F32, kind="ExternalInput")
    featp_d = nc.dram_tensor("featp", [128, KT, PPC], F32, kind="ExternalInput")
    featsamp_d = nc.dram_tensor("featsamp", [128, KT, SPC], F32, kind="ExternalInput")
    outt_d = nc.dram_tensor("outt", [128, NCLS, WO], F32, kind="ExternalInput")
    w1st_d = nc.dram_tensor("w1st", [128, KT, NCLS], F32, kind="ExternalInput")
    b1s_d = nc.dram_tensor("b1s", [NCLS, 1], F32, kind="ExternalInput")
    w2t_d = nc.dram_tensor("w2t", [128, KT, NCLS], F32, kind="ExternalInput")
    b2_d = nc.dram_tensor("b2", [NCLS, 1], F32, kind="ExternalInput")
    ut_d = nc.dram_tensor("ut", [64, 128], F32, kind="ExternalInput")    # U^T
    dtf_d = nc.dram_tensor("dtf", [128, 64], F32, kind="ExternalInput")   # D^T
    dtg_d = nc.dram_tensor("dtg", [128, 16], F32, kind="ExternalInput")   # D^T core's oh cols
    eye_d = nc.dram_tensor("eye", [128, 128], F32, kind="ExternalInput")

    pco_d = nc.dram_tensor("pco", [NCLS, HW], F32, kind="ExternalOutput")
    # [ow, slot, oh] on device (natural store); host relays out.
    sbool_d = nc.dram_tensor("sbool", [WO, SPC, HO], U8, kind="ExternalOutput")

    with ExitStack() as ctx:
        tc = ctx.enter_context(tile.TileContext(nc))
        consts = ctx.enter_context(tc.tile_pool(name="consts", bufs=1))
        persist = ctx.enter_context(tc.tile_pool(name="persist", bufs=1))
        fchunk = ctx.enter_context(tc.tile_pool(name="fchunk", bufs=3))
        sampbig = ctx.enter_context(tc.tile_pool(name="sampbig", bufs=2))
        work = ctx.enter_context(tc.tile_pool(name="work", bufs=2))
        ps = ctx.enter_context(tc.tile_pool(name="ps", bufs=3, space="PSUM"))
        acc = ctx.enter_context(tc.tile_pool(name="acc", bufs=4, space="PSUM"))
        dram = ctx.enter_context(tc.tile_pool(name="dram", bufs=1, space="DRAM"))

        for _rep in range(reps):
            # ---- constants to SBUF (all contiguous) ----
            w1st = consts.tile([128, KT, NCLS], F32)
            nc.sync.dma_start(w1st[:], w1st_d[:])
            w2t = consts.tile([128, KT, NCLS], F32)
            nc.sync.dma_start(w2t[:], w2t_d[:])
            b1s = consts.tile([NCLS, 1], F32)
            nc.sync.dma_start(b1s[:], b1s_d[:])
            b2 = consts.tile([NCLS, 1], F32)
            nc.sync.dma_start(b2[:], b2_d[:])
            ut = consts.tile([64, 128], F32)
            nc.sync.dma_start(ut[:], ut_d[:])
            dtf = consts.tile([128, 64], F32)
            nc.sync.dma_start(dtf[:], dtf_d[:])
            dtg = consts.tile([128, 16], F32)
            nc.sync.dma_start(dtg[:], dtg_d[:])
            eye = consts.tile([128, 128], F32)
            nc.sync.dma_start(eye[:], eye_d[:])
            featsamp = consts.tile([128, KT, SPC], F32)
            nc.sync.dma_start(featsamp[:], featsamp_d[:])
            featp = consts.tile([128, KT, PPC], F32)
            nc.sync.dma_start(featp[:], featp_d[:])
            if level < 1:
                continue

            # ---- f2 = w2 @ feat + b2 : [21, 4096] (feat streamed per chunk) ----
            f2 = persist.tile([NCLS, HW], F32)
            for qc in range(NQ):
                fc = fchunk.tile([128, KT, 512], F32, tag="fc")
                nc.sync.dma_start(fc[:], feat_d[:, :, qc * 512 : (qc + 1) * 512])
                t = ps.tile([NCLS, 512], F32, tag="ps")
                for kt in range(KT):
                    nc.tensor.matmul(
                        t[:], w2t[:, kt, :], fc[:, kt, :],
                        start=(kt == 0), stop=(kt == KT - 1),
                    )
                nc.scalar.activation(
                    f2[:, qc * 512 : (qc + 1) * 512], t[:], AF.Identity, bias=b2[:]
                )

            # ---- f1 on this core's p-columns: [21, 1024] (scale pre-folded) ----
            f1p = persist.tile([NCLS, PPC], F32)
            for qc in range(PPC // 512):
                t = ps.tile([NCLS, 512], F32, tag="ps")
                for kt in range(KT):
                    nc.tensor.matmul(
                        t[:], w1st[:, kt, :], featp[:, kt, qc * 512 : (qc + 1) * 512],
                        start=(kt == 0), stop=(kt == KT - 1),
                    )
                nc.scalar.activation(
                    f1p[:, qc * 512 : (qc + 1) * 512], t[:], AF.Identity, bias=b1s[:]
                )
            if level < 2:
                continue

            # ---- out_temp^T for this core's p rows -> ot_full [128, 8, 21] ----
            outT = persist.tile([128, NCLS, WO], F32)
            nc.sync.dma_start(outT[:], outt_d[:])
            # mm1: T1[oh, (c,w)] = dtg^T @ outT ; oh = this core's 16 rows
            T1 = persist.tile([16, NCLS * WO], F32)
            outT_f = outT[:].rearrange("p c w -> p (c w)")
            nfree = NCLS * WO  # 2688
            for chs in range(0, nfree, 512):
                sz = min(512, nfree - chs)
                t = ps.tile([16, 512], F32, tag="ps")
                nc.tensor.matmul(
                    t[:, :sz], dtg[:], outT_f[:, chs : chs + sz], start=True, stop=True
                )
                nc.scalar.activation(T1[:, chs : chs + sz], t[:, :sz], AF.Copy)
            # PE-transpose T1 -> T1T[w, oh, c] (oh-major free keeps later ops clean)
            T1v = T1[:].rearrange("p (c w) -> p c w", w=WO)
            T1T = persist.tile([128, 16, NCLS], F32)
            for c in range(NCLS):
                tp = ps.tile([128, 16], F32, tag="ps")
                nc.tensor.transpose(tp[:], T1v[:, c, :], eye[:16, :16])
                nc.vector.tensor_copy(T1T[:, :, c], tp[:])
            # mm2: T2[ow, ohr, c] = dtf^T @ T1T
            T2 = persist.tile([64, 16, NCLS], F32)
            t = ps.tile([64, 16 * NCLS], F32, tag="ps")
            nc.tensor.matmul(
                t[:], dtf[:], T1T[:].rearrange("p o c -> p (o c)"),
                start=True, stop=True,
            )
            nc.scalar.activation(T2[:].rearrange("p o c -> p (o c)"), t[:], AF.Copy)
            # ot_full[i, t, c]: p_local = t*128 + i = ohr*64 + ow
            #   even ohr -> partitions 0..63 (plain copy), odd -> 64..127 (DMA shift)
            ot_full = persist.tile([128, NBLK, NCLS], F32)
            nc.vector.tensor_copy(ot_full[0:64, :, :], T2[:, 0:16:2, :])
            nc.sync.dma_start(ot_full[64:128, :, :], T2[:, 1:16:2, :])
            if level < 3:
                continue

            # ---- phase A: E blocks (exp rows), row sums, folded weights ----
            E = persist.tile([128, NBLK, HW], BF16)
            otp = persist.tile([128, NBLK, NCLS], BF16)
            for b in range(NBLK):
                rs = work.tile([128, NQ], F32, tag="rs")
                for qc in range(NQ):
                    t = ps.tile([128, 512], F32, tag="ps")
                    nc.tensor.matmul(
                        t[:],
                        f1p[:, b * 128 : (b + 1) * 128],
                        f2[:, qc * 512 : (qc + 1) * 512],
                        start=True, stop=True,
                    )
                    nc.scalar.activation(
                        E[:, b, qc * 512 : (qc + 1) * 512], t[:], AF.Exp,
                        accum_out=rs[:, qc : qc + 1],
                    )
                rtot = work.tile([128, 1], F32, tag="rtot")
                nc.vector.reduce_sum(rtot[:], rs[:], axis=mybir.AxisListType.X)
                rcp = work.tile([128, 1], F32, tag="rcp")
                nc.vector.reciprocal(rcp[:], rtot[:])
                nc.vector.tensor_scalar_mul(otp[:, b, :], ot_full[:, b, :], rcp[:])
            if level < 4:
                continue

            # ---- phase B: partial corr_out[21, 4096] = sum_b otp_b^T @ E_b ----
            for qc in range(NQ):
                a = acc.tile([NCLS, 512], F32, tag="acc")
                for b in range(NBLK):
                    nc.tensor.matmul(
                        a[:], otp[:, b, :], E[:, b, qc * 512 : (qc + 1) * 512],
                        start=(b == 0), stop=(b == NBLK - 1),
                    )
                co = work.tile([NCLS, 512], F32, tag="co")
                nc.scalar.activation(co[:], a[:], AF.Copy)
                nc.sync.dma_start(pco_d[:, qc * 512 : (qc + 1) * 512], co[:])
            if level < 5:
                continue

            # ---- sampled rows: E_samp[32, 4096] (unnormalized softmax rows) ----
            fsp = ps.tile([NCLS, SPC], F32, tag="ps")
            for kt in range(KT):
                nc.tensor.matmul(
                    fsp[:], w1st[:, kt, :], featsamp[:, kt, :],
                    start=(kt == 0), stop=(kt == KT - 1),
                )
            f1s = persist.tile([NCLS, SPC], F32)
            nc.scalar.activation(f1s[:], fsp[:], AF.Identity, bias=b1s[:])
            Es = sampbig.tile([SPC, HW], F32, tag="sb")
            for qc in range(NQ):
                t = ps.tile([SPC, 512], F32, tag="ps")
                nc.tensor.matmul(
                    t[:], f1s[:], f2[:, qc * 512 : (qc + 1) * 512],
                    start=True, stop=True,
                )
                nc.scalar.activation(Es[:, qc * 512 : (qc + 1) * 512], t[:], AF.Exp)

            # rows -> images: De[s, h, w] -> cmT[h, (s,w)], split over 8 queues
            De = dram.tile([SPC, HW], F32)
            nc.sync.dma_start(De[:], Es[:])
            cmT = sampbig.tile([64, SPC, 64], F32, tag="sb")
            De_v = De[:].rearrange("s (h w) -> h s w", w=64)
            for g in range(8):
                nc.sync.dma_start(
                    cmT[8 * g : 8 * (g + 1), :, :], De_v[8 * g : 8 * (g + 1)]
                )
            # rows-interp: R1[o, (s,w)] = ut^T @ cmT
            R1 = sampbig.tile([128, SPC * 64], F32, tag="sb")
            cmT_f = cmT[:].rearrange("h s w -> h (s w)")
            for chs in range(0, SPC * 64, 512):
                t = ps.tile([128, 512], F32, tag="ps")
                nc.tensor.matmul(
                    t[:], ut[:], cmT_f[:, chs : chs + 512], start=True, stop=True
                )
                nc.scalar.activation(R1[:, chs : chs + 512], t[:], AF.Copy)
            # PE-transpose per-sample [128(o), 64(w)] -> [64(w), 128(o)]
            R1v = R1[:].rearrange("p (s w) -> p s w", w=64)
            R1T = sampbig.tile([64, SPC, 128], F32, tag="sb")
            for s in range(SPC):
                tp = ps.tile([64, 128], F32, tag="ps")
                nc.tensor.transpose(tp[:], R1v[:, s, :], eye[:])
                nc.vector.tensor_copy(R1T[:, s, :], tp[:])
            # cols-interp: R2[ow, (s, oh)] = ut^T @ R1T
            R2 = sampbig.tile([128, SPC, 128], F32, tag="sb")
            R1T_f = R1T[:].rearrange("p s o -> p (s o)")
            R2_f = R2[:].rearrange("p s o -> p (s o)")
            for chs in range(0, SPC * 128, 512):
                t = ps.tile([128, 512], F32, tag="ps")
                nc.tensor.matmul(
                    t[:], ut[:], R1T_f[:, chs : chs + 512], start=True, stop=True
                )
                nc.scalar.activation(R2_f[:, chs : chs + 512], t[:], AF.Copy)

            # min/max per sample image, threshold at (mn+mx)/2
            Amin = work.tile([128, SPC], F32, tag="amin")
            nc.vector.tensor_reduce(
                Amin[:], R2[:], axis=mybir.AxisListType.X, op=ALU.min
            )
            Amax = work.tile([128, SPC], F32, tag="amax")
            nc.vector.tensor_reduce(
                Amax[:], R2[:], axis=mybir.AxisListType.X, op=ALU.max
            )
            tmn = ps.tile([SPC, 128], F32, tag="ps")
            nc.tensor.transpose(tmn[:], Amin[:], eye[:])
            AminT = work.tile([SPC, 128], F32, tag="amint")
            nc.vector.tensor_copy(AminT[:], tmn[:])
            tmx = ps.tile([SPC, 128], F32, tag="ps")
            nc.tensor.transpose(tmx[:], Amax[:], eye[:])
            AmaxT = work.tile([SPC, 128], F32, tag="amaxt")
            nc.vector.tensor_copy(AmaxT[:], tmx[:])
            mn = work.tile([SPC, 1], F32, tag="mn")
            nc.vector.tensor_reduce(
                mn[:], AminT[:], axis=mybir.AxisListType.X, op=ALU.min
            )
            mx = work.tile([SPC, 1], F32, tag="mx")
            nc.vector.tensor_reduce(
                mx[:], AmaxT[:], axis=mybir.AxisListType.X, op=ALU.max
            )
            thr = work.tile([SPC, 1], F32, tag="thr")
            nc.vector.tensor_add(thr[:], mn[:], mx[:])
            nc.vector.tensor_scalar_mul(thr[:], thr[:], 0.5)
            # [32,1] -> [1,32] -> all partitions via ones outer product
            tthr = ps.tile([1, SPC], F32, tag="ps")
            nc.tensor.transpose(tthr[:], thr[:], eye[:SPC, :SPC])
            thr_row = work.tile([1, SPC], F32, tag="thrrow")
            nc.vector.tensor_copy(thr_row[:], tthr[:])
            ones = consts.tile([1, 128], F32)
            nc.vector.memset(ones[:], 1.0)
            tb = ps.tile([128, SPC], F32, tag="ps")
            nc.tensor.matmul(tb[:], ones[:], thr_row[:], start=True, stop=True)
            thr_sb = work.tile([128, SPC], F32, tag="thrsb")
            nc.scalar.activation(thr_sb[:], tb[:], AF.Copy)
            B8 = work.tile([128, SPC, 128], U8, tag="b8")
            for s in range(SPC):
                nc.vector.tensor_scalar(
                    B8[:, s, :], R2[:, s, :], thr_sb[:, s : s + 1], None, op0=ALU.is_gt
                )
            nc.sync.dma_start(sbool_d[:], B8[:])

    nc.compile()
    return nc


_NC_CACHE = {}


def _get_nc(reps: int = 1, level: int = 5):
    key = (reps, level)
    if key not in _NC_CACHE:
        _NC_CACHE[key] = _build_nc(reps, level)
    return _NC_CACHE[key]


def _tile_k(a):
    """[C, X] -> [128, KT, X] host pre-tiling of the contraction dim."""
    return np.ascontiguousarray(a.reshape(KT, 128, -1).transpose(1, 0, 2))


def prepare_in_maps(feature_in, out, w1, b1, w2, b2, index):
    feature_in = np.ascontiguousarray(np.asarray(feature_in, np.float32))
    out_np = np.ascontiguousarray(np.asarray(out, np.float32))
    w1 = np.asarray(w1, np.float32)
    b1 = np.asarray(b1, np.float32)
    w2 = np.asarray(w2, np.float32)
    b2 = np.asarray(b2, np.float32)
    idx = np.asarray(index).astype(np.int64)

    scale = np.float32(1.0) / np.sqrt(np.float32(NCLS))
    U = _resize_matrix(HO, H)        # [128, 64] upsample
    D = _resize_matrix(H, HO)        # [64, 128] downsample
    ut = np.ascontiguousarray(U.T)   # [64, 128]
    dtf = np.ascontiguousarray(D.T)  # [128, 64]
    eye = np.eye(128, dtype=np.float32)
    w1sT = _tile_k((w1 * scale).T)   # [128, 2, 21]
    b1s = np.ascontiguousarray((b1 * scale).reshape(NCLS, 1))
    w2T = _tile_k(w2.T)
    b2c = np.ascontiguousarray(b2.reshape(NCLS, 1))

    in_maps = []
    for c in range(NCORES):
        n, g = divmod(c, GROUPS)
        fi = feature_in[n].reshape(C, HW)
        in_maps.append(
            {
                "feat": _tile_k(fi),
                "featp": _tile_k(fi[:, g * PPC : (g + 1) * PPC]),
                "featsamp": _tile_k(fi[:, idx[g * SPC : (g + 1) * SPC]]),
                # [H, c, w] layout so the device load is contiguous
                "outt": np.ascontiguousarray(out_np[n].transpose(1, 0, 2)),
                "w1st": w1sT,
                "b1s": b1s,
                "w2t": w2T,
                "b2": b2c,
                "ut": ut,
                "dtf": dtf,
                "dtg": np.ascontiguousarray(dtf[:, g * 16 : (g + 1) * 16]),
                "eye": eye,
            }
        )
    return in_maps


def _assemble(res):
    corr_out = np.zeros((N, NCLS, HW), np.float32)
    cmo = np.zeros((N, M, HO, WO), np.uint8)
    for c in range(NCORES):
        n, g = divmod(c, GROUPS)
        corr_out[n] += res[c]["pco"]
        # sbool is [ow, slot, oh] -> [slot, oh, ow]
        cmo[n, g * SPC : (g + 1) * SPC] = res[c]["sbool"].transpose(1, 2, 0)
    corr_map_out = cmo.astype(bool)
    return corr_map_out, corr_out.reshape(N, NCLS, H, W)


def kernel(feature_in, out, w1, b1, w2, b2, index):
    in_maps = prepare_in_maps(feature_in, out, w1, b1, w2, b2, index)
    res = run_bass_kernel_spmd(_get_nc(), in_maps, list(range(NCORES))).results
    return _assemble(res)
